# revision 9
# baseline (speedup 1.0000x reference)
"""RWKV-style CausalEventModel kernel for 8 Trainium2 NeuronCores.

Strategy (zero cross-core communication):
  - Data-parallel over batch (B=4) x 2-way sequence split per batch = 8 cores.
  - Each core runs the FULL model on M=1040 tokens in channel-major layout
    ([D partitions, tokens free]).  The second-half core starts W=32 tokens
    early with zero initial WKV state; the per-channel decay makes the
    missing-prefix contribution negligible by the output region.

V3 rework (vs the V2 baseline at ~1.60 ms):
  - The residual h lives in ONE [P, KD, M] float32r tile; LN stats matmuls
    read it directly (f32r rhs at full PE rate), killing the bf16 pre-copies.
  - All per-j elementwise groups are merged into single 3D-AP instructions
    ([P, KD, cn]) so bf16 TT runs at the 2x packed rate with one instruction
    of fixed cost instead of four.
  - mixes use TS (4x mode, per-j scalar) + merged TT instead of 1x STTs.
  - LN apply is fused into the z production: z = (h - mean_b) * rstd_b,
    two merged TTs, no separate CAST.
  - Per-layer emission order software-pipelines the two halves so the PE
    chews CM projections of half 0 while the DVE runs the WKV recurrence of
    half 1, and vice versa; GpSimd absorbs the h^2 / z work of half 1.
  - mean/rstd broadcast: one paired PSUM matmul pair + ONE 3D ACT copy.
"""
import numpy as np
import ml_dtypes

B, T, E, D, F, L, OUT = 4, 2048, 4, 512, 2048, 8, 512
P = 128
KD = D // P          # 4
KF = F // P          # 16
W_WARM = 32
M = (T + W_WARM) // 2        # 1040 tokens per core
S_SPLIT = M                  # first-half output rows
HALVES = [(0, 512), (512, M - 512)]          # token half-blocks per core
HW = 576                     # PSUM out-tile width
HS = 528                     # local (per-half) SBUF tile width
N_CORES = 8
EPS = 1e-5

_CACHE = {}


def _mm_slices(cn):
    out = [(0, min(512, cn))]
    if cn > 512:
        out.append((512, cn - 512))
    return out


def _build_bass():
    import concourse.bass as bass  # noqa: F401
    import concourse.bacc as bacc
    import concourse.mybir as mybir
    import concourse.tile as tile
    from contextlib import ExitStack

    f32 = mybir.dt.float32
    f32r = mybir.dt.float32r
    b16 = mybir.dt.bfloat16
    AF = mybir.ActivationFunctionType
    OP = mybir.AluOpType

    nc = bacc.Bacc("TRN2", target_bir_lowering=False, debug=False)

    # ---------------- DRAM tensors ----------------
    xT_d = nc.dram_tensor("xT", [P, M], b16, kind="ExternalInput")
    wemb_d = nc.dram_tensor("wemb", [P, D], b16, kind="ExternalInput")
    inv_d = nc.dram_tensor("inv", [P, 12], f32, kind="ExternalInput")
    wk_d = nc.dram_tensor("wk", [L, KD, P, D], b16, kind="ExternalInput")
    wv_d = nc.dram_tensor("wv", [L, KD, P, D], b16, kind="ExternalInput")
    wr_d = nc.dram_tensor("wr", [L, KD, P, D], b16, kind="ExternalInput")
    wo_d = nc.dram_tensor("wo", [L, KD, P, D], b16, kind="ExternalInput")
    wck_d = nc.dram_tensor("wck", [L, KD, P, F], b16, kind="ExternalInput")
    wcv_d = nc.dram_tensor("wcv", [L, KF, P, D], b16, kind="ExternalInput")
    wcr_d = nc.dram_tensor("wcr", [L, KD, P, D], b16, kind="ExternalInput")
    whead_d = nc.dram_tensor("whead", [KD, P, OUT], b16, kind="ExternalInput")
    tmv_d = nc.dram_tensor("tmv", [L, P, 32], f32, kind="ExternalInput")
    cmv_d = nc.dram_tensor("cmv", [L, P, 28], f32, kind="ExternalInput")
    headb_d = nc.dram_tensor("headb", [P, KD], f32, kind="ExternalInput")
    out_d = nc.dram_tensor("outT", [KD, P, M], f32, kind="ExternalOutput")

    ctx = ExitStack()
    tc = ctx.enter_context(tile.TileContext(nc))
    sb = ctx.enter_context(tc.tile_pool(name="sb", bufs=1))
    vp = ctx.enter_context(tc.tile_pool(name="vp", bufs=2))
    wp = ctx.enter_context(tc.tile_pool(name="wp", bufs=1))
    hp = ctx.enter_context(tc.tile_pool(name="hp", bufs=2))   # per-half tiles
    pp = ctx.enter_context(tc.tile_pool(name="pp", bufs=2, space="PSUM"))

    # ---- persistent tiles ----
    h = sb.tile([P, KD, M], f32r, name="h", tag="h")
    z = sb.tile([P, KD, 2 + M], b16, name="z", tag="z")
    ms = sb.tile([P, 2, M], b16, name="ms", tag="ms")       # 0=mean 1=rstd bcast
    sc = sb.tile([P, KD, 1044], b16, name="sc", tag="sc")
    scB = sb.tile([P, KD, 1044], b16, name="scB", tag="scB")
    den = sb.tile([P, KD, HS], f32, name="den", tag="den")
    ones_b = sb.tile([P, 1], b16, name="ones_b", tag="ones_b")
    nc.vector.memset(ones_b, 1.0)
    ones_f32 = sb.tile([P, 1], f32, name="ones_f32", tag="ones_f32")
    nc.vector.memset(ones_f32, 1.0)
    ones_r = sb.tile([P, 1], f32r, name="ones_r", tag="ones_r")
    nc.vector.tensor_copy(out=ones_r, in_=ones_f32)
    ones_row = sb.tile([1, P], b16, name="ones_row", tag="ones_row")
    nc.vector.memset(ones_row, 1.0)
    eps_col = sb.tile([P, 1], f32, name="eps_col", tag="eps_col")
    nc.vector.memset(eps_col, EPS)
    nc.vector.memset(z[:, :, 1:2], 0.0)
    nc.vector.memset(sc[:, :, 0:1], 0.0)
    nc.vector.memset(scB[:, :, 0:1], 0.0)

    def halftile(tag, w=HS, dtype=b16, kd=KD, bufs=None):
        if bufs is None:
            return hp.tile([P, kd, w], dtype, name=tag, tag=tag)
        return hp.tile([P, kd, w], dtype, name=tag, tag=tag, bufs=bufs)

    # ---------------- LN helpers ----------------
    def stats(c0, cn, sq_eng):
        """Per-token mean/rstd of h[:, :, c0:c0+cn] -> ms (bf16 bcast)."""
        sq = halftile("sq", bufs=1)
        if sq_eng == "act":
            nc.scalar.activation(out=sq[:, :, :cn], in_=h[:, :, c0:c0 + cn],
                                 func=AF.Square)
        else:
            nc.gpsimd.tensor_tensor(sq[:, :, :cn], h[:, :, c0:c0 + cn],
                                    h[:, :, c0:c0 + cn], OP.mult)
        for (s0, sn) in _mm_slices(cn):
            a0 = c0 + s0
            sl = slice(a0, a0 + sn)
            srA = hp.tile([1, HW], f32, name="srA", tag="srA", bufs=1)
            srB = hp.tile([1, HW], f32, name="srB", tag="srB", bufs=1)
            rb = hp.tile([1, 2, HS], b16, name="rb", tag="rb", bufs=1)
            sum_ps = pp.tile([1, 512], f32, name="sum_ps", tag="st")
            sq_ps = pp.tile([1, 512], f32, name="sq_ps", tag="st")
            for j in range(KD):
                nc.tensor.matmul(sum_ps[0:1, :sn], lhsT=ones_r,
                                 rhs=h[:, j, a0:a0 + sn],
                                 start=(j == 0), stop=(j == KD - 1))
            for j in range(KD):
                nc.tensor.matmul(sq_ps[0:1, :sn], lhsT=ones_b,
                                 rhs=sq[:, j, s0:s0 + sn],
                                 start=(j == 0), stop=(j == KD - 1))
            nc.scalar.activation(out=srB[0:1, :sn], in_=sum_ps[0:1, :sn],
                                 func=AF.Square, scale=1.0 / D)
            nc.vector.scalar_tensor_tensor(out=srA[0:1, :sn],
                                           in0=sq_ps[0:1, :sn],
                                           scalar=1.0 / D, in1=srB[0:1, :sn],
                                           op0=OP.mult, op1=OP.subtract)
            nc.scalar.activation(out=srA[0:1, :sn], in_=srA[0:1, :sn],
                                 func=AF.Sqrt, bias=eps_col[0:1, :])
            nc.vector.reciprocal_approx_fast(out=srB[0:1, :sn],
                                             in_=srA[0:1, :sn])
            with nc.allow_low_precision(reason="per-token mean/rstd in bf16"):
                nc.vector.tensor_copy(out=rb[0:1, 1, :sn], in_=srB[0:1, :sn])
                nc.scalar.activation(out=rb[0:1, 0, :sn], in_=sum_ps[0:1, :sn],
                                     func=AF.Copy, scale=1.0 / D)
            bcm = pp.tile([P, HW], f32, name="bcm_ps", tag="mm", bufs=3)
            nc.tensor.matmul(bcm[:, 0:sn], lhsT=ones_row, rhs=rb[0:1, 0, :sn],
                             start=True, stop=True)
            bcr = pp.tile([P, HW], f32, name="bcr_ps", tag="mm", bufs=3)
            nc.tensor.matmul(bcr[:, 0:sn], lhsT=ones_row,
                             rhs=rb[0:1, 1, :sn], start=True, stop=True)
            nc.scalar.activation(out=ms[:, 0, sl], in_=bcm[:, :sn],
                                 func=AF.Copy)
            nc.scalar.activation(out=ms[:, 1, sl], in_=bcr[:, :sn],
                                 func=AF.Copy)

    def zchain(c0, cn, eng):
        """z = (h - mean_b) * rstd_b into the shared z tile (bf16)."""
        e = nc.vector if eng == "v" else nc.gpsimd
        zs = z[:, :, 2 + c0:2 + c0 + cn]
        mb = ms[:, 0:1, c0:c0 + cn].to_broadcast([P, KD, cn])
        rs = ms[:, 1:2, c0:c0 + cn].to_broadcast([P, KD, cn])
        e.tensor_tensor(zs, h[:, :, c0:c0 + cn], mb, OP.subtract)
        e.tensor_tensor(zs, zs, rs, OP.mult)

    def mixes(c0, cn, vec_t, cols):
        """xk = z + (mix-1)*(z - z_sh) per projection; returns out tiles."""
        d = halftile("dd", bufs=1)
        zc = z[:, :, 2 + c0:2 + c0 + cn]
        zsh = z[:, :, 1 + c0:1 + c0 + cn]
        nc.vector.tensor_tensor(d[:, :, :cn], zc, zsh, OP.subtract)
        outs = []
        for (tag, col) in cols:
            o = halftile(tag)
            for j in range(KD):
                nc.vector.tensor_scalar(out=o[:, j, :cn], in0=d[:, j, :cn],
                                        scalar1=vec_t[:, col + j:col + j + 1],
                                        scalar2=None, op0=OP.mult)
            nc.vector.tensor_tensor(o[:, :, :cn], o[:, :, :cn], zc, OP.add)
            outs.append(o)
        return outs

    def proj(rhs_t, w_t, cn, epilogue, nm=KD, nk=KD):
        for m in range(nm):
            ps = pp.tile([P, HW], f32, name="proj_ps", tag="mm", bufs=3)
            for (s0, sn) in _mm_slices(cn):
                for kj in range(nk):
                    nc.tensor.matmul(
                        ps[:, s0:s0 + sn],
                        lhsT=w_t[:, kj, m * P:(m + 1) * P],
                        rhs=rhs_t[:, kj, s0:s0 + sn],
                        start=(kj == 0), stop=(kj == nk - 1))
            epilogue(m, ps)

    # ---------------- embedding ----------------
    # xt borrows the kf slot (dead until the first CM phase)
    kfflat = halftile("kf", kd=KF, bufs=1).rearrange("p a b -> p (a b)")
    xt = kfflat[:, :M]
    nc.gpsimd.dma_start(out=xt, in_=xT_d[:, :])
    wemb_t = kfflat[:, M:M + D]
    nc.gpsimd.dma_start(out=wemb_t, in_=wemb_d[:, :])
    inv_t = sb.tile([P, 12], f32, name="inv_t", tag="inv_t")
    nc.gpsimd.dma_start(out=inv_t, in_=inv_d[:, :])
    headb_t = sb.tile([P, KD], f32, name="headb_t", tag="headb_t")
    nc.gpsimd.dma_start(out=headb_t, in_=headb_d[:, :])

    for (c0, cn) in HALVES:
        for m in range(KD):
            ps = pp.tile([P, HW], f32, name="emb_ps", tag="mm", bufs=3)
            for (s0, sn) in _mm_slices(cn):
                nc.tensor.matmul(ps[:, s0:s0 + sn],
                                 lhsT=wemb_t[:, m * P:(m + 1) * P],
                                 rhs=xt[:, c0 + s0:c0 + s0 + sn],
                                 start=True, stop=True)
            nc.scalar.activation(out=h[:, m, c0:c0 + cn], in_=ps[:, :cn],
                                 func=AF.Identity, bias=inv_t[:, m:m + 1])

    # ln_in: normalize then h = z*w + b
    for (c0, cn) in HALVES:
        stats(c0, cn, "act")
    for i, (c0, cn) in enumerate(HALVES):
        zchain(c0, cn, "v" if i == 0 else "g")
    for (c0, cn) in HALVES:
        for j in range(KD):
            nc.vector.tensor_scalar(out=h[:, j, c0:c0 + cn],
                                    in0=z[:, j, 2 + c0:2 + c0 + cn],
                                    scalar1=inv_t[:, 4 + j:5 + j],
                                    scalar2=inv_t[:, 8 + j:9 + j],
                                    op0=OP.mult, op1=OP.add)

    # ---------------- layers ----------------
    def tm_front_a(li):
        """TM weight DMAs + H0 stats for layer li.  Emitted right after the
        previous layer's cm_back(H0) so the PE/ACT have stats work ready
        while the previous layer's H1 wkv/out chain drains."""
        w = {"tmv": vp.tile([P, 32], f32, name=f"tmv{li}", tag="tmv")}
        nc.sync.dma_start(out=w["tmv"], in_=tmv_d[li])
        for nm, dr in (("wk", wk_d), ("wv", wv_d), ("wr", wr_d), ("wo", wo_d)):
            w[nm] = wp.tile([P, KD, D], b16, name=f"{nm}{li}", tag=nm)
            nc.sync.dma_start(out=w[nm], in_=dr[li].rearrange("k p d -> p k d"))
        c0, cn = HALVES[0]
        stats(c0, cn, "act")
        return w

    def tm_front_b(w):
        """H0 z + mixes (DVE) — emitted after the previous layer's CM-H1
        front so the h1-critical DVE ops stay ahead in the queue."""
        c0, cn = HALVES[0]
        zchain(c0, cn, "v")
        w["mix0"] = mixes(c0, cn, w["tmv"],
                          [("xkz", 0), ("xvz", 4), ("xrz", 8)])
        return w

    def tm_projs(cn, mix, w):
        """k/v/r projections + epilogues for one half."""
        tmv_t = w["tmv"]
        ek = halftile("ek")
        vv = halftile("vv")
        rr = halftile("rr")

        def k_epi(m, ps):
            nc.scalar.activation(out=ek[:, m, :cn], in_=ps[:, :cn],
                                 func=AF.Exp, bias=tmv_t[:, 20 + m:21 + m])

        def v_epi(m, ps):
            nc.scalar.activation(out=vv[:, m, :cn], in_=ps[:, :cn],
                                 func=AF.Identity,
                                 bias=tmv_t[:, 24 + m:25 + m])

        def r_epi(m, ps):
            nc.scalar.activation(out=rr[:, m, :cn], in_=ps[:, :cn],
                                 func=AF.Tanh, scale=0.5,
                                 bias=tmv_t[:, 28 + m:29 + m])

        proj(mix[0], w["wk"], cn, k_epi)
        proj(mix[1], w["wv"], cn, v_epi)
        proj(mix[2], w["wr"], cn, r_epi)
        return (ek, vv, rr, mix[0], mix[1], mix[2])

    def tm_wkv(c0, cn, hs, tmv_t):
        ek, vv, rr, xkz, xvz, xrz = hs
        ekv = xvz     # v-mix fully consumed by the v projection
        nc.vector.tensor_tensor(ekv[:, :, :cn], ek[:, :, :cn],
                                vv[:, :, :cn], OP.mult)
        for j in range(KD):
            ew = tmv_t[:, 12 + j:13 + j].to_broadcast([P, cn])
            init = 0.0 if c0 == 0 else sc[:, j, c0:c0 + 1]
            nc.vector.tensor_tensor_scan(out=sc[:, j, 1 + c0:1 + c0 + cn],
                                         data0=ew, data1=ekv[:, j, :cn],
                                         initial=init,
                                         op0=OP.mult, op1=OP.add)
        for j in range(KD):
            ew = tmv_t[:, 12 + j:13 + j].to_broadcast([P, cn])
            init = 0.0 if c0 == 0 else scB[:, j, c0:c0 + 1]
            nc.vector.tensor_tensor_scan(out=scB[:, j, 1 + c0:1 + c0 + cn],
                                         data0=ew, data1=ek[:, j, :cn],
                                         initial=init,
                                         op0=OP.mult, op1=OP.add)
        num = vv      # v values fully consumed by ekv
        for j in range(KD):
            nc.vector.scalar_tensor_tensor(out=num[:, j, :cn],
                                           in0=ekv[:, j, :cn],
                                           scalar=tmv_t[:, 16 + j:17 + j],
                                           in1=sc[:, j, c0:c0 + cn],
                                           op0=OP.mult, op1=OP.add)
        for j in range(KD):
            nc.vector.scalar_tensor_tensor(out=den[:, j, :cn],
                                           in0=ek[:, j, :cn],
                                           scalar=tmv_t[:, 16 + j:17 + j],
                                           in1=scB[:, j, c0:c0 + cn],
                                           op0=OP.mult, op1=OP.add)
        nc.vector.reciprocal_approx_fast(out=den[:, :, :cn],
                                         in_=den[:, :, :cn])
        rr2 = ek      # ek fully consumed by num/den/scans
        nc.vector.scalar_tensor_tensor(out=rr2[:, :, :cn],
                                       in0=rr[:, :, :cn], scalar=1.0,
                                       in1=den[:, :, :cn],
                                       op0=OP.add, op1=OP.mult)
        rwkv = xrz    # r-mix fully consumed by the r projection
        nc.vector.tensor_tensor(rwkv[:, :, :cn], num[:, :, :cn],
                                rr2[:, :, :cn], OP.mult)
        return rwkv

    def tm_out(c0, cn, rwkv, wo_t):
        cs = slice(c0, c0 + cn)

        def o_epi(m, ps):
            nc.vector.tensor_tensor(h[:, m, cs], h[:, m, cs], ps[:, :cn],
                                    OP.add)

        proj(rwkv, wo_t, cn, o_epi)

    def cm_front(c0, cn, sq_eng, z_eng, cmv_t, wcr_t):
        """stats + z + mixes + wcr/rf for one half."""
        stats(c0, cn, sq_eng)
        zchain(c0, cn, z_eng)
        xkc, xrc = mixes(c0, cn, cmv_t, [("xkz", 0), ("xrz", 4)])
        rf = halftile("xvz")

        def rf_epi(m, ps):
            nc.scalar.activation(out=rf[:, m, :cn], in_=ps[:, :cn],
                                 func=AF.Tanh, scale=0.5,
                                 bias=cmv_t[:, 8 + m:9 + m])

        proj(xrc, wcr_t, cn, rf_epi)
        return xkc, rf

    def cm_back(c0, cn, xkc, rf, cmv_t, wck_t, wcv_t):
        """wck + relu^2 + wcv + h update for one half."""
        cs = slice(c0, c0 + cn)
        kf = halftile("kf", kd=KF, bufs=1)
        for fo in range(KF):
            kfp = pp.tile([P, HW], f32, name="kfp", tag="mm", bufs=3)
            for (s0, sn) in _mm_slices(cn):
                for kj in range(KD):
                    nc.tensor.matmul(
                        kfp[:, s0:s0 + sn],
                        lhsT=wck_t[:, kj, fo * P:(fo + 1) * P],
                        rhs=xkc[:, kj, s0:s0 + sn],
                        start=(kj == 0), stop=(kj == KD - 1))
            nc.scalar.activation(out=kf[:, fo, :cn], in_=kfp[:, :cn],
                                 func=AF.Relu,
                                 bias=cmv_t[:, 12 + fo:13 + fo])
        nc.scalar.activation(out=kf[:, :, :cn], in_=kf[:, :, :cn],
                             func=AF.Square)
        t2 = halftile("t2", dtype=f32r)

        def wv_epi(m, ps):
            nc.vector.scalar_tensor_tensor(out=t2[:, m, :cn],
                                           in0=rf[:, m, :cn], scalar=1.0,
                                           in1=ps[:, :cn],
                                           op0=OP.add, op1=OP.mult)

        proj(kf, wcv_t, cn, wv_epi, nk=KF)
        nc.gpsimd.dma_start(out=h[:, :, cs], in_=t2[:, :, :cn],
                            accum_op=OP.add)

    whead_t = None
    cur = tm_front_b(tm_front_a(0))
    for li in range(L):
        tmv_t = cur["tmv"]
        # H1 stats/z on GpSimd while the PE starts H0 projections
        stats(*HALVES[1], "gps")
        zchain(*HALVES[1], "g")
        hs0 = tm_projs(HALVES[0][1], cur["mix0"], cur)
        mix1 = mixes(*HALVES[1], tmv_t, [("xkz", 0), ("xvz", 4), ("xrz", 8)])
        hs1 = tm_projs(HALVES[1][1], mix1, cur)
        rwkv0 = tm_wkv(*HALVES[0], hs0, tmv_t)
        tm_out(*HALVES[0], rwkv0, cur["wo"])

        cmv_t = vp.tile([P, 28], f32, name=f"cmv{li}", tag="cmv")
        nc.sync.dma_start(out=cmv_t, in_=cmv_d[li])
        wcr_t = wp.tile([P, KD, D], b16, name=f"wcr{li}", tag="wcr")
        nc.sync.dma_start(out=wcr_t, in_=wcr_d[li].rearrange("k p d -> p k d"))
        wck_t = wp.tile([P, KD, F], b16, name=f"wck{li}", tag="wck")
        nc.sync.dma_start(out=wck_t, in_=wck_d[li].rearrange("k p d -> p k d"))
        wcv_t = wp.tile([P, KF, D], b16, name=f"wcv{li}", tag="wcv")
        nc.sync.dma_start(out=wcv_t, in_=wcv_d[li].rearrange("k p d -> p k d"))

        # pipelined tail: CM-H0 front before wkv(H1) so the PE fills the
        # wkv(H1) window with wck(H0); next layer's TM-H0 front overlaps
        # the CM-H1 back half.
        xkc0, rf0 = cm_front(*HALVES[0], "act", "v", cmv_t, wcr_t)
        rwkv1 = tm_wkv(*HALVES[1], hs1, tmv_t)
        cm_back(*HALVES[0], xkc0, rf0, cmv_t, wck_t, wcv_t)
        if li + 1 < L:
            nxt = tm_front_a(li + 1)
        else:
            whead_t = wp.tile([P, KD, OUT], b16, name="whead_t", tag="wcr")
            nc.sync.dma_start(out=whead_t,
                              in_=whead_d.rearrange("k p d -> p k d"))
            stats(*HALVES[0], "act")
            nxt = None
        tm_out(*HALVES[1], rwkv1, cur["wo"])
        xkc1, rf1 = cm_front(*HALVES[1], "gps", "g", cmv_t, wcr_t)
        if nxt is not None:
            nxt = tm_front_b(nxt)
        else:
            zchain(*HALVES[0], "v")
        cm_back(*HALVES[1], xkc1, rf1, cmv_t, wck_t, wcv_t)
        cur = nxt

    # ---------------- final LN + head ----------------
    stats(*HALVES[1], "gps")
    zchain(*HALVES[1], "g")
    for (c0, cn) in HALVES:
        ho = halftile("t2", dtype=f32r)

        def head_epi(m, ps):
            nc.scalar.activation(out=ho[:, m, :cn], in_=ps[:, :cn],
                                 func=AF.Identity, bias=headb_t[:, m:m + 1])
            nc.sync.dma_start(out=out_d[m][:, c0:c0 + cn],
                              in_=ho[:, m, :cn].bitcast(f32))

        for m in range(KD):
            ps = pp.tile([P, HW], f32, name="head_ps", tag="mm", bufs=3)
            for (s0, sn) in _mm_slices(cn):
                for kj in range(KD):
                    nc.tensor.matmul(
                        ps[:, s0:s0 + sn],
                        lhsT=whead_t[:, kj, m * P:(m + 1) * P],
                        rhs=z[:, kj, 2 + c0 + s0:2 + c0 + s0 + sn],
                        start=(kj == 0), stop=(kj == KD - 1))
            head_epi(m, ps)

    ctx.close()
    nc.compile()
    return nc


def _pack_cols(vec, kd=KD):
    """[kd*P] -> [P, kd] so that column j holds channels j*P..(j+1)*P-1."""
    return np.ascontiguousarray(vec.reshape(kd, P).T)


def _prep_inputs(inputs):
    bf16 = ml_dtypes.bfloat16
    f32 = np.float32
    inp = {k: np.asarray(v, dtype=f32) for k, v in inputs.items()}

    shared = {}
    wemb_p = np.zeros((P, D), f32)
    wemb_p[:E] = inp["emb_w"]
    shared["wemb"] = wemb_p.astype(bf16)
    shared["inv"] = np.concatenate(
        [_pack_cols(inp["emb_b"]), _pack_cols(inp["ln_in_w"]),
         _pack_cols(inp["ln_in_b"])], axis=1).astype(f32)

    def fold(w_vec, mat):
        return (w_vec[:, None] * mat)

    wk = np.stack([fold(inp["ln0_w"][i], inp["tm_wk"][i]) for i in range(L)])
    # 0.5 folded into Wv: sigmoid(r) = 0.5*(tanh(r/2)+1), the 0.5 rides on v
    wv = np.stack([0.5 * fold(inp["ln0_w"][i], inp["tm_wv"][i])
                   for i in range(L)])
    wr = np.stack([fold(inp["ln0_w"][i], inp["tm_wr"][i]) for i in range(L)])
    wo = inp["tm_wo"]
    wck = np.stack([fold(inp["ln1_w"][i], inp["cm_wk"][i]) for i in range(L)])
    wcr = np.stack([fold(inp["ln1_w"][i], inp["cm_wr"][i]) for i in range(L)])
    wcv = 0.5 * inp["cm_wv"]

    shared["wk"] = wk.reshape(L, KD, P, D).astype(bf16)
    shared["wv"] = wv.reshape(L, KD, P, D).astype(bf16)
    shared["wr"] = wr.reshape(L, KD, P, D).astype(bf16)
    shared["wo"] = wo.reshape(L, KD, P, D).astype(bf16)
    shared["wck"] = wck.reshape(L, KD, P, F).astype(bf16)
    shared["wcv"] = wcv.reshape(L, KF, P, D).astype(bf16)
    shared["wcr"] = wcr.reshape(L, KD, P, D).astype(bf16)
    shared["whead"] = (inp["ln_out_w"][:, None] * inp["head_w"]).reshape(
        KD, P, OUT).astype(bf16)
    shared["headb"] = _pack_cols(inp["ln_out_b"] @ inp["head_w"]).astype(f32)

    tmv = np.zeros((L, P, 32), f32)
    cmv = np.zeros((L, P, 28), f32)
    for i in range(L):
        ew = np.exp(-np.exp(inp["tm_decay"][i]))
        tmv[i, :, 0:4] = _pack_cols(inp["tm_mix_k"][i] - 1.0)
        tmv[i, :, 4:8] = _pack_cols(inp["tm_mix_v"][i] - 1.0)
        tmv[i, :, 8:12] = _pack_cols(inp["tm_mix_r"][i] - 1.0)
        tmv[i, :, 12:16] = _pack_cols(ew)
        tmv[i, :, 16:20] = _pack_cols(np.exp(inp["tm_first"][i]))
        tmv[i, :, 20:24] = _pack_cols(inp["ln0_b"][i] @ inp["tm_wk"][i])
        tmv[i, :, 24:28] = _pack_cols(0.5 * (inp["ln0_b"][i] @ inp["tm_wv"][i]))
        tmv[i, :, 28:32] = _pack_cols(0.5 * (inp["ln0_b"][i] @ inp["tm_wr"][i]))
        cmv[i, :, 0:4] = _pack_cols(inp["cm_mix_k"][i] - 1.0)
        cmv[i, :, 4:8] = _pack_cols(inp["cm_mix_r"][i] - 1.0)
        cmv[i, :, 8:12] = _pack_cols(0.5 * (inp["ln1_b"][i] @ inp["cm_wr"][i]))
        cmv[i, :, 12:28] = _pack_cols(inp["ln1_b"][i] @ inp["cm_wk"][i], kd=KF)
    shared["tmv"] = tmv
    shared["cmv"] = cmv

    in_maps = []
    x = inp["x"]
    for c in range(N_CORES):
        b, half = c // 2, c % 2
        t0 = 0 if half == 0 else T - M
        x_sl = np.zeros((P, M), f32)
        x_sl[:E] = x[b, t0:t0 + M].T
        m = dict(shared)
        m["xT"] = x_sl.astype(bf16)
        in_maps.append(m)
    return in_maps


TRACE = False  # set by test harness to capture an NTFF profile


def kernel(**inputs):
    from concourse import bass_utils

    if "nc" not in _CACHE:
        _CACHE["nc"] = _build_bass()
    nc = _CACHE["nc"]
    in_maps = _prep_inputs(inputs)
    res = bass_utils.run_bass_kernel_spmd(nc, in_maps, core_ids=list(range(N_CORES)),
                                          trace=TRACE)
    _CACHE["last_res"] = res
    out = np.zeros((B, T, OUT), np.float32)
    for c in range(N_CORES):
        b, half = c // 2, c % 2
        oT = res.results[c]["outT"].reshape(D, M)  # [channels, tokens]
        o = np.ascontiguousarray(oT.T)             # [tokens, channels]
        if half == 0:
            out[b, :S_SPLIT] = o[:S_SPLIT]
        else:
            out[b, S_SPLIT:] = o[M - (T - S_SPLIT):]
    return out


# revision 10
# speedup vs baseline: 1.2452x; 1.2452x over previous
"""RWKV-style CausalEventModel kernel for 8 Trainium2 NeuronCores.

Strategy (zero cross-core communication):
  - Data-parallel over batch (B=4) x 2-way sequence split per batch = 8 cores.
  - Each core runs the FULL model on M=1040 tokens in channel-major layout
    ([D partitions, tokens free]).  The second-half core starts W=32 tokens
    early with zero initial WKV state; the per-channel decay makes the
    missing-prefix contribution negligible by the output region.

V3 rework (vs the V2 baseline at ~1.60 ms):
  - The residual h lives in ONE [P, KD, M] float32r tile; LN stats matmuls
    read it directly (f32r rhs at full PE rate), killing the bf16 pre-copies.
  - All per-j elementwise groups are merged into single 3D-AP instructions
    ([P, KD, cn]) so bf16 TT runs at the 2x packed rate with one instruction
    of fixed cost instead of four.
  - mixes use TS (4x mode, per-j scalar) + merged TT instead of 1x STTs.
  - LN apply is fused into the z production: z = (h - mean_b) * rstd_b,
    two merged TTs, no separate CAST.
  - Per-layer emission order software-pipelines the two halves so the PE
    chews CM projections of half 0 while the DVE runs the WKV recurrence of
    half 1, and vice versa; GpSimd absorbs the h^2 / z work of half 1.
  - mean/rstd broadcast: one paired PSUM matmul pair + ONE 3D ACT copy.
"""
import numpy as np
import ml_dtypes

B, T, E, D, F, L, OUT = 4, 2048, 4, 512, 2048, 8, 512
P = 128
KD = D // P          # 4
KF = F // P          # 16
W_WARM = 32
M = (T + W_WARM) // 2        # 1040 tokens per core
S_SPLIT = M                  # first-half output rows
HALVES = [(0, 512), (512, M - 512)]          # token half-blocks per core
HW = 576                     # PSUM out-tile width
HS = 528                     # local (per-half) SBUF tile width
N_CORES = 8
EPS = 1e-5

_CACHE = {}


def _mm_slices(cn):
    out = [(0, min(512, cn))]
    if cn > 512:
        out.append((512, cn - 512))
    return out


def _build_bass():
    import concourse.bass as bass  # noqa: F401
    import concourse.bacc as bacc
    import concourse.mybir as mybir
    import concourse.tile as tile
    from contextlib import ExitStack

    f32 = mybir.dt.float32
    f32r = mybir.dt.float32r
    b16 = mybir.dt.bfloat16
    AF = mybir.ActivationFunctionType
    OP = mybir.AluOpType

    nc = bacc.Bacc("TRN2", target_bir_lowering=False, debug=False)

    # ---------------- DRAM tensors ----------------
    xT_d = nc.dram_tensor("xT", [P, M], b16, kind="ExternalInput")
    wemb_d = nc.dram_tensor("wemb", [P, D], b16, kind="ExternalInput")
    inv_d = nc.dram_tensor("inv", [P, 12], f32, kind="ExternalInput")
    wk_d = nc.dram_tensor("wk", [L, KD, P, D], b16, kind="ExternalInput")
    wv_d = nc.dram_tensor("wv", [L, KD, P, D], b16, kind="ExternalInput")
    wr_d = nc.dram_tensor("wr", [L, KD, P, D], b16, kind="ExternalInput")
    wo_d = nc.dram_tensor("wo", [L, KD, P, D], b16, kind="ExternalInput")
    wck_d = nc.dram_tensor("wck", [L, KD, P, F], b16, kind="ExternalInput")
    wcv_d = nc.dram_tensor("wcv", [L, KF, P, D], b16, kind="ExternalInput")
    wcr_d = nc.dram_tensor("wcr", [L, KD, P, D], b16, kind="ExternalInput")
    whead_d = nc.dram_tensor("whead", [KD, P, OUT], b16, kind="ExternalInput")
    tmv_d = nc.dram_tensor("tmv", [L, P, 32], f32, kind="ExternalInput")
    cmv_d = nc.dram_tensor("cmv", [L, P, 28], f32, kind="ExternalInput")
    headb_d = nc.dram_tensor("headb", [P, KD], f32, kind="ExternalInput")
    out_d = nc.dram_tensor("outT", [KD, P, M], f32, kind="ExternalOutput")

    ctx = ExitStack()
    tc = ctx.enter_context(tile.TileContext(nc))
    sb = ctx.enter_context(tc.tile_pool(name="sb", bufs=1))
    vp = ctx.enter_context(tc.tile_pool(name="vp", bufs=2))
    wp = ctx.enter_context(tc.tile_pool(name="wp", bufs=1))
    hp = ctx.enter_context(tc.tile_pool(name="hp", bufs=2))   # per-half tiles
    pp = ctx.enter_context(tc.tile_pool(name="pp", bufs=2, space="PSUM"))

    # ---- persistent tiles ----
    h = sb.tile([P, KD, M], f32r, name="h", tag="h")
    z = sb.tile([P, KD, 2 + M], b16, name="z", tag="z")
    ms = sb.tile([P, 2, M], b16, name="ms", tag="ms")       # 0=mean 1=rstd bcast
    sc = sb.tile([P, KD, 1044], b16, name="sc", tag="sc")
    scB = sb.tile([P, KD, 1044], b16, name="scB", tag="scB")
    den = sb.tile([P, KD, HS], f32, name="den", tag="den")
    ones_b = sb.tile([P, 1], b16, name="ones_b", tag="ones_b")
    nc.vector.memset(ones_b, 1.0)
    ones_f32 = sb.tile([P, 1], f32, name="ones_f32", tag="ones_f32")
    nc.vector.memset(ones_f32, 1.0)
    ones_r = sb.tile([P, 1], f32r, name="ones_r", tag="ones_r")
    nc.vector.tensor_copy(out=ones_r, in_=ones_f32)
    ones_row = sb.tile([1, P], b16, name="ones_row", tag="ones_row")
    nc.vector.memset(ones_row, 1.0)
    eps_col = sb.tile([P, 1], f32, name="eps_col", tag="eps_col")
    nc.vector.memset(eps_col, EPS)
    nc.vector.memset(z[:, :, 1:2], 0.0)
    nc.vector.memset(sc[:, :, 0:1], 0.0)
    nc.vector.memset(scB[:, :, 0:1], 0.0)

    def halftile(tag, w=HS, dtype=b16, kd=KD, bufs=None):
        if bufs is None:
            return hp.tile([P, kd, w], dtype, name=tag, tag=tag)
        return hp.tile([P, kd, w], dtype, name=tag, tag=tag, bufs=bufs)

    # ---------------- LN helpers ----------------
    def stats(c0, cn, sq_eng):
        """Per-token mean/rstd of h[:, :, c0:c0+cn] -> ms (bf16 bcast)."""
        sq = halftile("sq", bufs=1)
        if sq_eng == "act":
            nc.scalar.activation(out=sq[:, :, :cn], in_=h[:, :, c0:c0 + cn],
                                 func=AF.Square)
        else:
            nc.gpsimd.tensor_tensor(sq[:, :, :cn], h[:, :, c0:c0 + cn],
                                    h[:, :, c0:c0 + cn], OP.mult)
        for (s0, sn) in _mm_slices(cn):
            a0 = c0 + s0
            sl = slice(a0, a0 + sn)
            srA = hp.tile([1, HW], f32, name="srA", tag="srA", bufs=1)
            srB = hp.tile([1, HW], f32, name="srB", tag="srB", bufs=1)
            rb = hp.tile([1, 2, HS], b16, name="rb", tag="rb", bufs=1)
            sum_ps = pp.tile([1, 512], f32, name="sum_ps", tag="st")
            sq_ps = pp.tile([1, 512], f32, name="sq_ps", tag="st")
            for j in range(KD):
                nc.tensor.matmul(sum_ps[0:1, :sn], lhsT=ones_r,
                                 rhs=h[:, j, a0:a0 + sn],
                                 start=(j == 0), stop=(j == KD - 1))
            for j in range(KD):
                nc.tensor.matmul(sq_ps[0:1, :sn], lhsT=ones_b,
                                 rhs=sq[:, j, s0:s0 + sn],
                                 start=(j == 0), stop=(j == KD - 1))
            nc.scalar.activation(out=srB[0:1, :sn], in_=sum_ps[0:1, :sn],
                                 func=AF.Square, scale=1.0 / D)
            nc.vector.scalar_tensor_tensor(out=srA[0:1, :sn],
                                           in0=sq_ps[0:1, :sn],
                                           scalar=1.0 / D, in1=srB[0:1, :sn],
                                           op0=OP.mult, op1=OP.subtract)
            nc.scalar.activation(out=srA[0:1, :sn], in_=srA[0:1, :sn],
                                 func=AF.Sqrt, bias=eps_col[0:1, :])
            nc.vector.reciprocal_approx_fast(out=srB[0:1, :sn],
                                             in_=srA[0:1, :sn])
            with nc.allow_low_precision(reason="per-token mean/rstd in bf16"):
                nc.vector.tensor_copy(out=rb[0:1, 1, :sn], in_=srB[0:1, :sn])
                nc.scalar.activation(out=rb[0:1, 0, :sn], in_=sum_ps[0:1, :sn],
                                     func=AF.Copy, scale=1.0 / D)
            bcm = pp.tile([P, HW], f32, name="bcm_ps", tag="mm", bufs=3)
            nc.tensor.matmul(bcm[:, 0:sn], lhsT=ones_row, rhs=rb[0:1, 0, :sn],
                             start=True, stop=True)
            bcr = pp.tile([P, HW], f32, name="bcr_ps", tag="mm", bufs=3)
            nc.tensor.matmul(bcr[:, 0:sn], lhsT=ones_row,
                             rhs=rb[0:1, 1, :sn], start=True, stop=True)
            nc.scalar.activation(out=ms[:, 0, sl], in_=bcm[:, :sn],
                                 func=AF.Copy)
            nc.scalar.activation(out=ms[:, 1, sl], in_=bcr[:, :sn],
                                 func=AF.Copy)

    def zchain(c0, cn, eng):
        """z = (h - mean_b) * rstd_b into the shared z tile (bf16)."""
        e = nc.vector if eng == "v" else nc.gpsimd
        zs = z[:, :, 2 + c0:2 + c0 + cn]
        mb = ms[:, 0:1, c0:c0 + cn].to_broadcast([P, KD, cn])
        rs = ms[:, 1:2, c0:c0 + cn].to_broadcast([P, KD, cn])
        e.tensor_tensor(zs, h[:, :, c0:c0 + cn], mb, OP.subtract)
        e.tensor_tensor(zs, zs, rs, OP.mult)

    def mixes(c0, cn, vec_t, cols):
        """xk = z + (mix-1)*(z - z_sh) per projection; returns out tiles."""
        d = halftile("dd", bufs=1)
        zc = z[:, :, 2 + c0:2 + c0 + cn]
        zsh = z[:, :, 1 + c0:1 + c0 + cn]
        nc.vector.tensor_tensor(d[:, :, :cn], zc, zsh, OP.subtract)
        outs = []
        for (tag, col) in cols:
            o = halftile(tag)
            for j in range(KD):
                nc.vector.tensor_scalar(out=o[:, j, :cn], in0=d[:, j, :cn],
                                        scalar1=vec_t[:, col + j:col + j + 1],
                                        scalar2=None, op0=OP.mult)
            nc.vector.tensor_tensor(o[:, :, :cn], o[:, :, :cn], zc, OP.add)
            ka(o[:, 0, 0:64])
            outs.append(o)
        return outs

    def ka(src_ap):
        """HAM keep-alive: tiny matmul on a just-produced DVE result so the
        PE activity window never sees a >3.4us idle stretch (keeps the clock
        at 8/8)."""
        ka_ps = pp.tile([1, 512], f32, name="ka_ps", tag="st")
        nc.tensor.matmul(ka_ps[0:1, :64], lhsT=ones_b, rhs=src_ap,
                         start=True, stop=True)

    def proj(rhs_t, w_t, cn, epilogue, nm=KD, nk=KD):
        for m in range(nm):
            ps = pp.tile([P, HW], f32, name="proj_ps", tag="mm", bufs=3)
            for (s0, sn) in _mm_slices(cn):
                for kj in range(nk):
                    nc.tensor.matmul(
                        ps[:, s0:s0 + sn],
                        lhsT=w_t[:, kj, m * P:(m + 1) * P],
                        rhs=rhs_t[:, kj, s0:s0 + sn],
                        start=(kj == 0), stop=(kj == nk - 1))
            epilogue(m, ps)

    # ---------------- embedding ----------------
    # xt borrows the kf slot (dead until the first CM phase)
    kfflat = halftile("kf", kd=KF, bufs=1).rearrange("p a b -> p (a b)")
    xt = kfflat[:, :M]
    nc.gpsimd.dma_start(out=xt, in_=xT_d[:, :])
    wemb_t = kfflat[:, M:M + D]
    nc.gpsimd.dma_start(out=wemb_t, in_=wemb_d[:, :])
    inv_t = sb.tile([P, 12], f32, name="inv_t", tag="inv_t")
    nc.gpsimd.dma_start(out=inv_t, in_=inv_d[:, :])
    headb_t = sb.tile([P, KD], f32, name="headb_t", tag="headb_t")
    nc.gpsimd.dma_start(out=headb_t, in_=headb_d[:, :])

    for (c0, cn) in HALVES:
        for m in range(KD):
            ps = pp.tile([P, HW], f32, name="emb_ps", tag="mm", bufs=3)
            for (s0, sn) in _mm_slices(cn):
                nc.tensor.matmul(ps[:, s0:s0 + sn],
                                 lhsT=wemb_t[:, m * P:(m + 1) * P],
                                 rhs=xt[:, c0 + s0:c0 + s0 + sn],
                                 start=True, stop=True)
            nc.scalar.activation(out=h[:, m, c0:c0 + cn], in_=ps[:, :cn],
                                 func=AF.Identity, bias=inv_t[:, m:m + 1])

    # ln_in: normalize then h = z*w + b
    for (c0, cn) in HALVES:
        stats(c0, cn, "act")
    for i, (c0, cn) in enumerate(HALVES):
        zchain(c0, cn, "v" if i == 0 else "g")
    for (c0, cn) in HALVES:
        for j in range(KD):
            nc.vector.tensor_scalar(out=h[:, j, c0:c0 + cn],
                                    in0=z[:, j, 2 + c0:2 + c0 + cn],
                                    scalar1=inv_t[:, 4 + j:5 + j],
                                    scalar2=inv_t[:, 8 + j:9 + j],
                                    op0=OP.mult, op1=OP.add)

    # ---------------- layers ----------------
    def tm_front_a(li):
        """TM weight DMAs + H0 stats for layer li.  Emitted right after the
        previous layer's cm_back(H0) so the PE/ACT have stats work ready
        while the previous layer's H1 wkv/out chain drains."""
        w = {"tmv": vp.tile([P, 32], f32, name=f"tmv{li}", tag="tmv")}
        nc.sync.dma_start(out=w["tmv"], in_=tmv_d[li])
        for nm, dr in (("wk", wk_d), ("wv", wv_d), ("wr", wr_d), ("wo", wo_d)):
            w[nm] = wp.tile([P, KD, D], b16, name=f"{nm}{li}", tag=nm)
            nc.sync.dma_start(out=w[nm], in_=dr[li].rearrange("k p d -> p k d"))
        c0, cn = HALVES[0]
        stats(c0, cn, "act")
        return w

    def tm_front_b(w):
        """H0 z + mixes (DVE) — emitted after the previous layer's CM-H1
        front so the h1-critical DVE ops stay ahead in the queue."""
        c0, cn = HALVES[0]
        zchain(c0, cn, "v")
        w["mix0"] = mixes(c0, cn, w["tmv"],
                          [("xkz", 0), ("xvz", 4), ("xrz", 8)])
        return w

    def tm_projs(cn, mix, w):
        """k/v/r projections + epilogues for one half."""
        tmv_t = w["tmv"]
        ek = halftile("ek")
        vv = halftile("vv")
        rr = halftile("rr")

        def k_epi(m, ps):
            nc.scalar.activation(out=ek[:, m, :cn], in_=ps[:, :cn],
                                 func=AF.Exp, bias=tmv_t[:, 20 + m:21 + m])

        def v_epi(m, ps):
            nc.scalar.activation(out=vv[:, m, :cn], in_=ps[:, :cn],
                                 func=AF.Identity,
                                 bias=tmv_t[:, 24 + m:25 + m])

        def r_epi(m, ps):
            nc.scalar.activation(out=rr[:, m, :cn], in_=ps[:, :cn],
                                 func=AF.Tanh, scale=0.5,
                                 bias=tmv_t[:, 28 + m:29 + m])

        proj(mix[0], w["wk"], cn, k_epi)
        proj(mix[1], w["wv"], cn, v_epi)
        proj(mix[2], w["wr"], cn, r_epi)
        return (ek, vv, rr, mix[0], mix[1], mix[2])

    def tm_wkv(c0, cn, hs, tmv_t):
        ek, vv, rr, xkz, xvz, xrz = hs
        ekv = xvz     # v-mix fully consumed by the v projection
        nc.vector.tensor_tensor(ekv[:, :, :cn], ek[:, :, :cn],
                                vv[:, :, :cn], OP.mult)
        for j in range(KD):
            ew = tmv_t[:, 12 + j:13 + j].to_broadcast([P, cn])
            init = 0.0 if c0 == 0 else sc[:, j, c0:c0 + 1]
            nc.vector.tensor_tensor_scan(out=sc[:, j, 1 + c0:1 + c0 + cn],
                                         data0=ew, data1=ekv[:, j, :cn],
                                         initial=init,
                                         op0=OP.mult, op1=OP.add)
            ka(sc[:, j, 1 + c0:65 + c0])
        for j in range(KD):
            ew = tmv_t[:, 12 + j:13 + j].to_broadcast([P, cn])
            init = 0.0 if c0 == 0 else scB[:, j, c0:c0 + 1]
            nc.vector.tensor_tensor_scan(out=scB[:, j, 1 + c0:1 + c0 + cn],
                                         data0=ew, data1=ek[:, j, :cn],
                                         initial=init,
                                         op0=OP.mult, op1=OP.add)
            if j % 2 == 1:
                ka(scB[:, j, 1 + c0:65 + c0])
        num = vv      # v values fully consumed by ekv
        for j in range(KD):
            nc.vector.scalar_tensor_tensor(out=num[:, j, :cn],
                                           in0=ekv[:, j, :cn],
                                           scalar=tmv_t[:, 16 + j:17 + j],
                                           in1=sc[:, j, c0:c0 + cn],
                                           op0=OP.mult, op1=OP.add)
        for j in range(KD):
            nc.vector.scalar_tensor_tensor(out=den[:, j, :cn],
                                           in0=ek[:, j, :cn],
                                           scalar=tmv_t[:, 16 + j:17 + j],
                                           in1=scB[:, j, c0:c0 + cn],
                                           op0=OP.mult, op1=OP.add)
        nc.vector.reciprocal_approx_fast(out=den[:, :, :cn],
                                         in_=den[:, :, :cn])
        rr2 = ek      # ek fully consumed by num/den/scans
        nc.vector.scalar_tensor_tensor(out=rr2[:, :, :cn],
                                       in0=rr[:, :, :cn], scalar=1.0,
                                       in1=den[:, :, :cn],
                                       op0=OP.add, op1=OP.mult)
        rwkv = xrz    # r-mix fully consumed by the r projection
        nc.vector.tensor_tensor(rwkv[:, :, :cn], num[:, :, :cn],
                                rr2[:, :, :cn], OP.mult)
        ka(rwkv[:, 0, 0:64])
        return rwkv

    def tm_out(c0, cn, rwkv, wo_t):
        cs = slice(c0, c0 + cn)

        def o_epi(m, ps):
            nc.vector.tensor_tensor(h[:, m, cs], h[:, m, cs], ps[:, :cn],
                                    OP.add)

        proj(rwkv, wo_t, cn, o_epi)

    def cm_front(c0, cn, sq_eng, z_eng, cmv_t, wcr_t):
        """stats + z + mixes + wcr/rf for one half."""
        stats(c0, cn, sq_eng)
        zchain(c0, cn, z_eng)
        xkc, xrc = mixes(c0, cn, cmv_t, [("xkz", 0), ("xrz", 4)])
        rf = halftile("xvz")

        def rf_epi(m, ps):
            nc.scalar.activation(out=rf[:, m, :cn], in_=ps[:, :cn],
                                 func=AF.Tanh, scale=0.5,
                                 bias=cmv_t[:, 8 + m:9 + m])

        proj(xrc, wcr_t, cn, rf_epi)
        return xkc, rf

    def cm_back(c0, cn, xkc, rf, cmv_t, wck_t, wcv_t):
        """wck + relu^2 + wcv + h update for one half."""
        cs = slice(c0, c0 + cn)
        kf = halftile("kf", kd=KF, bufs=1)
        for fo in range(KF):
            kfp = pp.tile([P, HW], f32, name="kfp", tag="mm", bufs=3)
            for (s0, sn) in _mm_slices(cn):
                for kj in range(KD):
                    nc.tensor.matmul(
                        kfp[:, s0:s0 + sn],
                        lhsT=wck_t[:, kj, fo * P:(fo + 1) * P],
                        rhs=xkc[:, kj, s0:s0 + sn],
                        start=(kj == 0), stop=(kj == KD - 1))
            nc.scalar.activation(out=kf[:, fo, :cn], in_=kfp[:, :cn],
                                 func=AF.Relu,
                                 bias=cmv_t[:, 12 + fo:13 + fo])
        nc.scalar.activation(out=kf[:, :, :cn], in_=kf[:, :, :cn],
                             func=AF.Square)
        t2 = halftile("t2", dtype=f32r)

        def wv_epi(m, ps):
            nc.vector.scalar_tensor_tensor(out=t2[:, m, :cn],
                                           in0=rf[:, m, :cn], scalar=1.0,
                                           in1=ps[:, :cn],
                                           op0=OP.add, op1=OP.mult)

        proj(kf, wcv_t, cn, wv_epi, nk=KF)
        nc.gpsimd.dma_start(out=h[:, :, cs], in_=t2[:, :, :cn],
                            accum_op=OP.add)

    whead_t = None
    cur = tm_front_b(tm_front_a(0))
    for li in range(L):
        tmv_t = cur["tmv"]
        # H1 stats/z on GpSimd while the PE starts H0 projections
        stats(*HALVES[1], "gps")
        zchain(*HALVES[1], "g")
        hs0 = tm_projs(HALVES[0][1], cur["mix0"], cur)
        mix1 = mixes(*HALVES[1], tmv_t, [("xkz", 0), ("xvz", 4), ("xrz", 8)])
        hs1 = tm_projs(HALVES[1][1], mix1, cur)
        rwkv0 = tm_wkv(*HALVES[0], hs0, tmv_t)
        tm_out(*HALVES[0], rwkv0, cur["wo"])

        cmv_t = vp.tile([P, 28], f32, name=f"cmv{li}", tag="cmv")
        nc.sync.dma_start(out=cmv_t, in_=cmv_d[li])
        wcr_t = wp.tile([P, KD, D], b16, name=f"wcr{li}", tag="wcr")
        nc.sync.dma_start(out=wcr_t, in_=wcr_d[li].rearrange("k p d -> p k d"))
        wck_t = wp.tile([P, KD, F], b16, name=f"wck{li}", tag="wck")
        nc.sync.dma_start(out=wck_t, in_=wck_d[li].rearrange("k p d -> p k d"))
        wcv_t = wp.tile([P, KF, D], b16, name=f"wcv{li}", tag="wcv")
        nc.sync.dma_start(out=wcv_t, in_=wcv_d[li].rearrange("k p d -> p k d"))

        # pipelined tail: CM-H0 front before wkv(H1) so the PE fills the
        # wkv(H1) window with wck(H0); next layer's TM-H0 front overlaps
        # the CM-H1 back half.
        xkc0, rf0 = cm_front(*HALVES[0], "act", "v", cmv_t, wcr_t)
        rwkv1 = tm_wkv(*HALVES[1], hs1, tmv_t)
        cm_back(*HALVES[0], xkc0, rf0, cmv_t, wck_t, wcv_t)
        tm_out(*HALVES[1], rwkv1, cur["wo"])
        xkc1, rf1 = cm_front(*HALVES[1], "gps", "g", cmv_t, wcr_t)
        if li + 1 < L:
            nxt = tm_front_b(tm_front_a(li + 1))
        else:
            whead_t = wp.tile([P, KD, OUT], b16, name="whead_t", tag="wcr")
            nc.sync.dma_start(out=whead_t,
                              in_=whead_d.rearrange("k p d -> p k d"))
            stats(*HALVES[0], "act")
            zchain(*HALVES[0], "v")
            nxt = None
        cm_back(*HALVES[1], xkc1, rf1, cmv_t, wck_t, wcv_t)
        cur = nxt

    # ---------------- final LN + head ----------------
    stats(*HALVES[1], "gps")
    zchain(*HALVES[1], "g")
    for (c0, cn) in HALVES:
        ho = halftile("t2", dtype=f32r)

        def head_epi(m, ps):
            nc.scalar.activation(out=ho[:, m, :cn], in_=ps[:, :cn],
                                 func=AF.Identity, bias=headb_t[:, m:m + 1])
            nc.sync.dma_start(out=out_d[m][:, c0:c0 + cn],
                              in_=ho[:, m, :cn].bitcast(f32))

        for m in range(KD):
            ps = pp.tile([P, HW], f32, name="head_ps", tag="mm", bufs=3)
            for (s0, sn) in _mm_slices(cn):
                for kj in range(KD):
                    nc.tensor.matmul(
                        ps[:, s0:s0 + sn],
                        lhsT=whead_t[:, kj, m * P:(m + 1) * P],
                        rhs=z[:, kj, 2 + c0 + s0:2 + c0 + s0 + sn],
                        start=(kj == 0), stop=(kj == KD - 1))
            head_epi(m, ps)

    ctx.close()
    nc.compile()
    return nc


def _pack_cols(vec, kd=KD):
    """[kd*P] -> [P, kd] so that column j holds channels j*P..(j+1)*P-1."""
    return np.ascontiguousarray(vec.reshape(kd, P).T)


def _prep_inputs(inputs):
    bf16 = ml_dtypes.bfloat16
    f32 = np.float32
    inp = {k: np.asarray(v, dtype=f32) for k, v in inputs.items()}

    shared = {}
    wemb_p = np.zeros((P, D), f32)
    wemb_p[:E] = inp["emb_w"]
    shared["wemb"] = wemb_p.astype(bf16)
    shared["inv"] = np.concatenate(
        [_pack_cols(inp["emb_b"]), _pack_cols(inp["ln_in_w"]),
         _pack_cols(inp["ln_in_b"])], axis=1).astype(f32)

    def fold(w_vec, mat):
        return (w_vec[:, None] * mat)

    wk = np.stack([fold(inp["ln0_w"][i], inp["tm_wk"][i]) for i in range(L)])
    # 0.5 folded into Wv: sigmoid(r) = 0.5*(tanh(r/2)+1), the 0.5 rides on v
    wv = np.stack([0.5 * fold(inp["ln0_w"][i], inp["tm_wv"][i])
                   for i in range(L)])
    wr = np.stack([fold(inp["ln0_w"][i], inp["tm_wr"][i]) for i in range(L)])
    wo = inp["tm_wo"]
    wck = np.stack([fold(inp["ln1_w"][i], inp["cm_wk"][i]) for i in range(L)])
    wcr = np.stack([fold(inp["ln1_w"][i], inp["cm_wr"][i]) for i in range(L)])
    wcv = 0.5 * inp["cm_wv"]

    shared["wk"] = wk.reshape(L, KD, P, D).astype(bf16)
    shared["wv"] = wv.reshape(L, KD, P, D).astype(bf16)
    shared["wr"] = wr.reshape(L, KD, P, D).astype(bf16)
    shared["wo"] = wo.reshape(L, KD, P, D).astype(bf16)
    shared["wck"] = wck.reshape(L, KD, P, F).astype(bf16)
    shared["wcv"] = wcv.reshape(L, KF, P, D).astype(bf16)
    shared["wcr"] = wcr.reshape(L, KD, P, D).astype(bf16)
    shared["whead"] = (inp["ln_out_w"][:, None] * inp["head_w"]).reshape(
        KD, P, OUT).astype(bf16)
    shared["headb"] = _pack_cols(inp["ln_out_b"] @ inp["head_w"]).astype(f32)

    tmv = np.zeros((L, P, 32), f32)
    cmv = np.zeros((L, P, 28), f32)
    for i in range(L):
        ew = np.exp(-np.exp(inp["tm_decay"][i]))
        tmv[i, :, 0:4] = _pack_cols(inp["tm_mix_k"][i] - 1.0)
        tmv[i, :, 4:8] = _pack_cols(inp["tm_mix_v"][i] - 1.0)
        tmv[i, :, 8:12] = _pack_cols(inp["tm_mix_r"][i] - 1.0)
        tmv[i, :, 12:16] = _pack_cols(ew)
        tmv[i, :, 16:20] = _pack_cols(np.exp(inp["tm_first"][i]))
        tmv[i, :, 20:24] = _pack_cols(inp["ln0_b"][i] @ inp["tm_wk"][i])
        tmv[i, :, 24:28] = _pack_cols(0.5 * (inp["ln0_b"][i] @ inp["tm_wv"][i]))
        tmv[i, :, 28:32] = _pack_cols(0.5 * (inp["ln0_b"][i] @ inp["tm_wr"][i]))
        cmv[i, :, 0:4] = _pack_cols(inp["cm_mix_k"][i] - 1.0)
        cmv[i, :, 4:8] = _pack_cols(inp["cm_mix_r"][i] - 1.0)
        cmv[i, :, 8:12] = _pack_cols(0.5 * (inp["ln1_b"][i] @ inp["cm_wr"][i]))
        cmv[i, :, 12:28] = _pack_cols(inp["ln1_b"][i] @ inp["cm_wk"][i], kd=KF)
    shared["tmv"] = tmv
    shared["cmv"] = cmv

    in_maps = []
    x = inp["x"]
    for c in range(N_CORES):
        b, half = c // 2, c % 2
        t0 = 0 if half == 0 else T - M
        x_sl = np.zeros((P, M), f32)
        x_sl[:E] = x[b, t0:t0 + M].T
        m = dict(shared)
        m["xT"] = x_sl.astype(bf16)
        in_maps.append(m)
    return in_maps


TRACE = False  # set by test harness to capture an NTFF profile


def kernel(**inputs):
    from concourse import bass_utils

    if "nc" not in _CACHE:
        _CACHE["nc"] = _build_bass()
    nc = _CACHE["nc"]
    in_maps = _prep_inputs(inputs)
    res = bass_utils.run_bass_kernel_spmd(nc, in_maps, core_ids=list(range(N_CORES)),
                                          trace=TRACE)
    _CACHE["last_res"] = res
    out = np.zeros((B, T, OUT), np.float32)
    for c in range(N_CORES):
        b, half = c // 2, c % 2
        oT = res.results[c]["outT"].reshape(D, M)  # [channels, tokens]
        o = np.ascontiguousarray(oT.T)             # [tokens, channels]
        if half == 0:
            out[b, :S_SPLIT] = o[:S_SPLIT]
        else:
            out[b, S_SPLIT:] = o[M - (T - S_SPLIT):]
    return out
